# revision 30
# baseline (speedup 1.0000x reference)
"""Multi-head causal attention (B=4, S=2048, D=1024, H=16) on 8 Trainium2 cores.

Sharding: core c -> (batch b = c//2, head-half hh = c%2), i.e. each core computes
attention for one batch and 8 of the 16 heads, plus the partial output
projection against its row-shard of Wo. Host sums the per-batch core pair
(the Wo row-shard all-reduce) and transposes.

On-device layout (per core, all matmul operands bf16, accumulation fp32):
  - q/k projections produce qT/kT [head-pair 128, S] (features on partitions)
  - scores are computed transposed: S^T[t, s] tiles (keys on partitions) so
    exp() on ScalarE writes P^T directly, and softmax denominators come for
    free from a ones-column appended to V during the P^T @ V_aug matmul.
  - causal masking: tiles strictly above the diagonal are never computed;
    diagonal tiles get a 0/1 mask multiply post-exp.
  - the two heads of a pair live on disjoint partition rows (0-63 / 64-127),
    so their K=64 S^T matmuls execute concurrently in the PE array
    (row-group tiling); attention is pipelined over (pair, s-range) units.
  - attention is ScalarE(exp)-bound, so Q/K/V projection matmuls of later
    pairs are emitted as PE "filler" between score windows to hide them.
"""

import os
import sys
from contextlib import ExitStack

for _p in (
    "/opt/trn_rl_repo/concourse",
    "/root/.axon_site/_ro/trn_rl_repo/concourse",
):
    if os.path.isdir(_p) and _p not in sys.path:
        sys.path.append(_p)

import numpy as np
import ml_dtypes

BF16 = ml_dtypes.bfloat16

HD = 64          # head dim
NH = 8           # heads per core
G = NH // 2      # head-pair groups (2 heads -> 128 partitions)
EC = NH * HD // 128  # o^T feature chunks (=4)


def build_nc(S, D):
    import concourse.tile as tile
    from concourse import bacc, mybir

    f32 = mybir.dt.float32
    bf16 = mybir.dt.bfloat16
    Exp = mybir.ActivationFunctionType.Exp
    add = mybir.AluOpType.add
    mult = mybir.AluOpType.mult

    KC = D // 128    # contraction chunks over model dim
    ST = S // 128    # 128-token tiles
    SC = S // 512    # 512-token score groups
    NU = max(1, SC // 2)          # s-range units per head pair
    UW = (SC // NU) * 512         # unit width in columns

    nc = bacc.Bacc(None, target_bir_lowering=False)

    xq = nc.dram_tensor("xq", [D, S], bf16, kind="ExternalInput")
    xk = nc.dram_tensor("xk", [D, S], bf16, kind="ExternalInput")
    xv = nc.dram_tensor("xv", [D, S], bf16, kind="ExternalInput")
    wq = nc.dram_tensor("wq", [D, NH * HD], bf16, kind="ExternalInput")
    wk = nc.dram_tensor("wk", [D, NH * HD], bf16, kind="ExternalInput")
    wv = nc.dram_tensor("wv", [D, NH * HD], bf16, kind="ExternalInput")
    wo = nc.dram_tensor("wo", [NH * HD, D], bf16, kind="ExternalInput")
    bqd = nc.dram_tensor("bq", [128, G], f32, kind="ExternalInput")
    bkd = nc.dram_tensor("bk", [128, G], f32, kind="ExternalInput")
    bvd = nc.dram_tensor("bv", [128, NH, HD], f32, kind="ExternalInput")
    bod = nc.dram_tensor("bo", [128, D // 128], f32, kind="ExternalInput")
    maskd = nc.dram_tensor("mask", [128, 128], bf16, kind="ExternalInput")
    out = nc.dram_tensor("out", [D, S], f32, kind="ExternalOutput")

    with tile.TileContext(nc) as tc, ExitStack() as ctx:
        const_pool = ctx.enter_context(tc.tile_pool(name="const", bufs=1))
        wqk_pool = ctx.enter_context(tc.tile_pool(name="wqk", bufs=2))
        xpool = ctx.enter_context(tc.tile_pool(name="x", bufs=3))
        qk_pool = ctx.enter_context(tc.tile_pool(name="qk", bufs=1))
        v_pool = ctx.enter_context(tc.tile_pool(name="v", bufs=1))
        pt_pool = ctx.enter_context(tc.tile_pool(name="pt", bufs=1))
        o_pool = ctx.enter_context(tc.tile_pool(name="o", bufs=1))
        r_pool = ctx.enter_context(tc.tile_pool(name="r", bufs=2))
        ou_pool = ctx.enter_context(tc.tile_pool(name="ou", bufs=2))
        out_pool = ctx.enter_context(tc.tile_pool(name="outp", bufs=2))
        ps_mm = ctx.enter_context(tc.tile_pool(name="psmm", bufs=4, space="PSUM"))
        ps_st = ctx.enter_context(tc.tile_pool(name="psst", bufs=2, space="PSUM"))

        # ---- constants into SBUF (wv/wo DMAs deferred past the upfront proj) ----
        wv_sb = const_pool.tile([128, KC, NH * HD], bf16)
        wo_sb = const_pool.tile([128, EC, D], bf16)
        bq_sb = const_pool.tile([128, G], f32)
        nc.sync.dma_start(bq_sb[:], bqd[:])
        bk_sb = const_pool.tile([128, G], f32)
        nc.sync.dma_start(bk_sb[:], bkd[:])
        bv_sb = const_pool.tile([128, NH, HD], f32)
        nc.sync.dma_start(bv_sb[:], bvd[:])
        bo_sb = const_pool.tile([128, D // 128], f32)
        nc.sync.dma_start(bo_sb[:], bod[:])
        mask_sb = const_pool.tile([128, 128], bf16)
        nc.sync.dma_start(mask_sb[:], maskd[:])

        qT = qk_pool.tile([128, G, S], bf16, tag="qT")
        kT = qk_pool.tile([128, G, S], bf16, tag="kT")
        v_sb = v_pool.tile([128, ST, NH, HD + 1], bf16, tag="v")
        oT = o_pool.tile([128, EC, S], bf16, tag="oT")

        # ones columns of V_aug (softmax denominator trick)
        nc.vector.memset(v_sb[:, :, :, HD : HD + 1], 1.0)

        # ---- projection emitters (used upfront and as PE filler blocks) ----
        def load_xslice(xsrc, sc, name):
            """One batched DMA: all KC chunks of a 512-token column slice."""
            xt = xpool.tile([128, KC, 512], bf16, tag="xb", name=name)
            nc.sync.dma_start(
                xt[:],
                xsrc.rearrange("(kc p) s -> p kc s", p=128)[
                    :, :, sc * 512 : (sc + 1) * 512
                ],
            )
            return xt

        def kchain(psum_ap, lhs_of, rhs_of, n):
            for c in range(n):
                nc.tensor.matmul(
                    psum_ap, lhs_of(c), rhs_of(c),
                    start=(c == 0), stop=(c == n - 1),
                )

        def proj_qk_block(pname, xsrc, wsb, bsb, dst, gh, sc):
            """One s-chunk of the q or k projection for pair-half gh
            (pairs 2gh, 2gh+1): 1 DMA + 16 MMs + 2 bias copies."""
            xt = xpool_slot(xsrc, pname, sc)
            for i in range(2):
                psum = ps_mm.tile([128, 512], f32, tag="mm", name=f"p{pname}{gh}_{sc}_{i}")
                kchain(
                    psum[:],
                    lambda c, i=i: wsb[:, c, i * 128 : (i + 1) * 128],
                    lambda c: xt[:, c, :],
                    KC,
                )
                g = 2 * gh + i
                nc.vector.tensor_scalar_add(
                    dst[:, g, sc * 512 : (sc + 1) * 512], psum[:], bsb[:, g : g + 1]
                )

        def xpool_slot(xsrc, pname, sc):
            return load_xslice(xsrc, sc, f"x{pname}_{sc}_{id(xsrc) % 97}")

        def proj_v_block(sc):
            """One s-chunk of the V projection: 1 DMA + 32 MMs + 4 bias adds,
            never more than 2 PSUM slots in flight."""
            xt = load_xslice(xv, sc, f"xv_{sc}")
            for half in range(2):
                for i in range(2):
                    psum = ps_mm.tile([128, NH, HD], f32, tag="mm", name=f"psv{sc}_{half}_{i}")
                    kchain(
                        psum[:],
                        lambda c, i=i, half=half: xt[
                            :, c, half * 256 + i * 128 : half * 256 + (i + 1) * 128
                        ],
                        lambda c: wv_sb[:, c, :],
                        KC,
                    )
                    sti = sc * 4 + half * 2 + i
                    nc.vector.tensor_tensor(
                        v_sb[:, sti, :, 0:HD], psum[:], bv_sb[:], add
                    )

        def make_ghalf_blocks(gh):
            """Filler thunks: q+k projection s-chunks for pairs 2gh, 2gh+1."""
            wqg = wqk_pool.tile([128, KC, 256], bf16, tag="wqk", name=f"wq_{gh}")
            wkg = wqk_pool.tile([128, KC, 256], bf16, tag="wqk", name=f"wk_{gh}")
            nc.sync.dma_start(
                wqg[:],
                wq[:, gh * 256 : (gh + 1) * 256].rearrange("(kc p) m -> p kc m", p=128),
            )
            nc.sync.dma_start(
                wkg[:],
                wk[:, gh * 256 : (gh + 1) * 256].rearrange("(kc p) m -> p kc m", p=128),
            )
            blocks = []
            for sc in range(SC):
                blocks.append(
                    lambda sc=sc: proj_qk_block("q", xq, wqg, bq_sb, qT, gh, sc)
                )
            for sc in range(SC):
                blocks.append(
                    lambda sc=sc: proj_qk_block("k", xk, wkg, bk_sb, kT, gh, sc)
                )
            return blocks

        # ---- unit geometry ----
        units = [(g, u) for g in range(G) for u in range(NU)]

        def unit_geom(u):
            s_lo, s_hi = u * UW, min((u + 1) * UW, S)
            ts, offs, cols = [], {}, 0
            for t in range(0, s_hi // 128):
                w = s_hi - max(t * 128, s_lo)
                if w <= 0:
                    continue
                ts.append(t)
                offs[t] = cols
                cols += w
            return s_lo, s_hi, ts, offs, cols

        def st_unit(g, u, filler):
            """S^T + exp + mask for both heads of pair g over unit u's columns.
            The two heads' K=64 matmuls use disjoint PE row groups and run
            concurrently. One filler block is emitted after each exp window to
            keep the PE busy while ScalarE drains."""
            s_lo, s_hi, ts, offs, cols = unit_geom(u)
            pts = []
            for j in range(2):
                pt = pt_pool.tile(
                    [128, cols], bf16, tag=f"ph{u}_{j}", name=f"pt{g}_{u}_{j}", bufs=1
                )
                pts.append(pt)
            chunks = []  # (t, s_from, win, win_off, wlen)
            pos = 0
            for t in ts:
                s_from = max(t * 128, s_lo)
                rem = s_hi - s_from
                while rem:
                    wlen = min(512 - pos % 512, rem)
                    chunks.append((t, s_from, pos // 1024, pos % 1024, wlen))
                    pos += wlen
                    s_from += wlen
                    rem -= wlen
            nwin = (pos + 1023) // 1024
            for w in range(nwin):
                wchunks = [c for c in chunks if c[2] == w]
                wcols = sum(c[4] for c in wchunks)
                gbase = 1024 * w
                stt = []
                for j in range(2):
                    st_t = ps_st.tile(
                        [128, 1024], f32, tag="st", name=f"st{g}_{u}_{w}_{j}"
                    )
                    stt.append(st_t)
                for t, s_from, _, woff, wlen in wchunks:
                    for j in range(2):
                        ro = j * HD
                        nc.tensor.matmul(
                            stt[j][:, woff : woff + wlen],
                            kT[ro : ro + HD, g, t * 128 : t * 128 + 128],
                            qT[ro : ro + HD, g, s_from : s_from + wlen],
                            start=True,
                            stop=True,
                        )
                for j in range(2):
                    nc.scalar.activation(
                        pts[j][:, gbase : gbase + wcols],
                        stt[j][:, 0:wcols],
                        Exp,
                        scale=1.0 / np.sqrt(HD),
                    )
                if filler is not None:
                    blk = filler()
                    if blk is not None:
                        blk()
            # causal mask on diagonal tiles (t starting inside this unit)
            for t in ts:
                if t * 128 >= s_lo:
                    for j in range(2):
                        nc.vector.tensor_tensor(
                            pts[j][:, offs[t] : offs[t] + 128],
                            pts[j][:, offs[t] : offs[t] + 128],
                            mask_sb[:],
                            mult,
                        )
            return pts

        def outproj_block(dc, schalf):
            scs = list(range(2 * schalf, min(2 * schalf + 2, SC)))
            ot = out_pool.tile(
                [128, len(scs), 512], f32, tag="ot", name=f"ot{dc}_{schalf}"
            )
            for k, sc in enumerate(scs):
                psum = ps_mm.tile([128, 512], f32, tag="mm", name=f"po{dc}_{schalf}_{k}")
                kchain(
                    psum[:],
                    lambda c: wo_sb[:, c, dc * 128 : (dc + 1) * 128],
                    lambda c, sc=sc: oT[:, c, sc * 512 : (sc + 1) * 512],
                    EC,
                )
                nc.vector.tensor_scalar_add(
                    ot[:, k, :], psum[:], bo_sb[:, dc : dc + 1]
                )
            nc.sync.dma_start(
                out[dc * 128 : (dc + 1) * 128, scs[0] * 512 : (scs[-1] + 1) * 512],
                ot[:],
            )

        def av_unit(g, u, pts):
            """O^T accumulation + normalization for both heads of pair g,
            s-groups of unit u (group-at-a-time: 2 PSUM slots in flight)."""
            s_lo, s_hi, ts, offs, cols = unit_geom(u)
            for g4 in range(s_lo // 512, s_hi // 512):
                for j in range(2):
                    h = 2 * g + j
                    ro = j * HD
                    av = ps_mm.tile(
                        [128, 512], f32, tag="mm", name=f"av{g}_{u}_{j}_{g4}"
                    )
                    tlist = [t for t in ts if t * 128 < (g4 + 1) * 512]
                    for ci, t in enumerate(tlist):
                        lo = max(g4 * 512, t * 128)
                        n = (g4 + 1) * 512 - lo
                        col = offs[t] + lo - max(t * 128, s_lo)
                        nc.tensor.matmul(
                            av[0 : HD + 1, lo - g4 * 512 : lo - g4 * 512 + n],
                            v_sb[:, t, h, :],
                            pts[j][:, col : col + n],
                            start=(ci == 0),
                            stop=(ci == len(tlist) - 1),
                        )
                    # evacuate PSUM quickly (frees the accumulator slot),
                    # then normalize off the critical path
                    osb = ou_pool.tile([HD + 1, 512], f32, tag="ou", name=f"ou_{g}_{u}_{j}_{g4}")
                    nc.vector.tensor_copy(osb[:], av[0 : HD + 1, :])
                    rs = r_pool.tile([128, 4], f32, tag="rs", name=f"rs_{g}_{u}_{j}_{g4}")
                    nc.sync.dma_start(rs[:], osb[HD : HD + 1, :])
                    rr = r_pool.tile([128, 4], f32, tag="rr", name=f"rr_{g}_{u}_{j}_{g4}")
                    nc.vector.reciprocal(rr[:], rs[:])
                    r1 = r_pool.tile([1, 512], f32, tag="r1x", name=f"r1_{g}_{u}_{j}_{g4}")
                    nc.sync.dma_start(r1[:], rr[:])
                    r64 = r_pool.tile(
                        [HD, 512], f32, tag="r64", name=f"r64_{g}_{u}_{j}_{g4}"
                    )
                    nc.gpsimd.partition_broadcast(r64[:], r1[:])
                    nc.vector.tensor_tensor(
                        oT[ro : ro + HD, g, g4 * 512 : (g4 + 1) * 512],
                        osb[0:HD, :],
                        r64[:],
                        mult,
                    )

        # ---- emission ----
        # Emission order defines data dependencies: all proj blocks for a pair
        # must be emitted before that pair's score matmuls, and a V block
        # before the first av_unit reading its v tiles. Blocks are otherwise
        # fed as PE filler between score windows so projections hide under the
        # ScalarE-bound attention phase.
        from collections import deque

        # pairs 0+1 q/k projection upfront. The first score unit only needs
        # the first half of the s-columns of q and the low key tiles of k, so
        # it is emitted between the two projection halves to start ScalarE
        # (exp) as early as possible.
        blocks0 = make_ghalf_blocks(0)
        nA = (min(UW, S) + 511) // 512
        partA = blocks0[0:nA] + blocks0[SC : SC + nA]
        partB = blocks0[nA:SC] + blocks0[SC + nA :]
        for blk in partA:
            blk()
        pts_first = st_unit(units[0][0], units[0][1], None)
        for blk in partB:
            blk()
        nc.sync.dma_start(wv_sb[:], wv.rearrange("(kc p) m -> p kc m", p=128))
        nc.sync.dma_start(wo_sb[:], wo.rearrange("(ec p) d -> p ec d", p=128))

        v_state = {"covered": 0}

        def vblk(sc):
            def f():
                proj_v_block(sc)
                v_state["covered"] += 1
            return f

        vpending = deque(vblk(sc) for sc in range(SC))
        pending = deque(make_ghalf_blocks(1)) if G > 2 else deque()
        if os.environ.get("NOFILL"):
            while vpending:
                vpending.popleft()()
            while pending:
                pending.popleft()()

        def take_filler():
            if vpending:
                return vpending.popleft()
            if pending:
                return pending.popleft()
            return None

        def prep_for_st(p):
            # pairs >= 2 are covered by the gh=1 blocks
            if p >= 2:
                while pending:
                    pending.popleft()()

        def prep_for_av(u):
            s_hi = min((u + 1) * UW, S)
            while v_state["covered"] * 512 < s_hi and vpending:
                vpending.popleft()()

        pts_next = pts_first
        for i, (g, u) in enumerate(units):
            pts_cur = pts_next
            if i + 1 < len(units):
                gn, un = units[i + 1]
                prep_for_st(gn)
                pts_next = st_unit(gn, un, take_filler)
            prep_for_av(u)
            av_unit(g, u, pts_cur)
            if NU > 1 and i == len(units) - 2:
                # left s-half of the output projection: its oT columns are
                # complete; run it while the last unit's exps drain
                for dc in range(D // 128):
                    outproj_block(dc, 0)
        while vpending:
            vpending.popleft()()
        while pending:
            pending.popleft()()

        # ---- output projection: out = (Wo_shard.T @ o^T) + bo/2 ----
        for schalf in range((SC + 1) // 2) if False else []:
            pass
        for dc in range(D // 128):
            for schalf in range(1, (SC + 1) // 2) if NU > 1 else range((SC + 1) // 2):
                outproj_block(dc, schalf)

    nc.compile()
    return nc


def core_inputs(queries, keys, values, Wq, bq, Wk, bk, Wv, bv, Wo, bo, b, hh):
    """Build the per-core input map (host-side sharding + bf16 cast)."""
    D = queries.shape[2]
    hs = slice(hh * NH, hh * NH + NH)

    def xt(x):
        return np.ascontiguousarray(x[b].astype(BF16).T)

    def wcat(W):
        return np.ascontiguousarray(
            np.transpose(W[hs], (1, 0, 2)).reshape(D, NH * HD).astype(BF16)
        )

    def bstack(bias):
        return np.ascontiguousarray(
            bias[hs].reshape(G, 128).T.astype(np.float32)
        )

    mask = np.triu(np.ones((128, 128), np.float32)).astype(BF16)
    return {
        "xq": xt(queries),
        "xk": xt(keys),
        "xv": xt(values),
        "wq": wcat(Wq),
        "wk": wcat(Wk),
        "wv": wcat(Wv),
        "wo": np.ascontiguousarray(Wo[hh * NH * HD : (hh + 1) * NH * HD].astype(BF16)),
        "bq": bstack(bq),
        "bk": bstack(bk),
        "bv": np.ascontiguousarray(
            np.broadcast_to(bv[hs].reshape(1, NH, HD), (128, NH, HD)).astype(np.float32)
        ),
        "bo": np.ascontiguousarray(
            (bo.reshape(D // 128, 128) / 2.0).T.astype(np.float32)
        ),
        "mask": mask,
    }


_NC_CACHE = {}


def _get_nc(S, D):
    key = (S, D)
    if key not in _NC_CACHE:
        _NC_CACHE[key] = build_nc(S, D)
    return _NC_CACHE[key]


def kernel(keys, queries, values, Wq, bq, Wk, bk, Wv, bv, Wo, bo, _trace=False):
    keys, queries, values = (np.asarray(a) for a in (keys, queries, values))
    Wq, bq, Wk, bk, Wv, bv, Wo, bo = (
        np.asarray(a) for a in (Wq, bq, Wk, bk, Wv, bv, Wo, bo)
    )
    B, S, D = queries.shape
    nc = _get_nc(S, D)

    in_maps = [
        core_inputs(queries, keys, values, Wq, bq, Wk, bk, Wv, bv, Wo, bo, c // 2, c % 2)
        for c in range(8)
    ]
    from concourse.bass_utils import run_bass_kernel_spmd

    res = run_bass_kernel_spmd(
        nc, in_maps, core_ids=list(range(8)), trace=_trace
    )
    kernel.last_result = res
    outs = [r["out"] for r in res.results]
    out = np.empty((B, S, D), np.float32)
    for b in range(B):
        out[b] = (outs[2 * b] + outs[2 * b + 1]).T
    return out
